# revision 21
# baseline (speedup 1.0000x reference)
"""Distributed AND-convolution (Dempster combination / FWHT-style) for 8 TRN2 cores.

out = mobius(zeta(m1) * zeta(m2)) over 24 bit-axes, L = 2^24.

Sharding: top 3 bits (h = k[23:21]) = core id. Per core per channel: 2^21
elements, SBUF (128, 16384): partition p = l[20:14], free f = l[13:0].

Slab pipelining: f-blocks of 512 are interleaved into 4 slabs
(slab s = blocks with blk%4 == s, blk = f>>9). Free-bit butterfly stages
j in {0..8, 11..13} are intra-slab; j in {9, 10} are cross-slab and run
full-tile at the start (fwd) / end (inv). Each slab flows independently:
  fwd stages -> zeta matmul (7 partition bits) -> AllToAll ->
  h-conv (zeta3/product/mobius3 matmuls on (c,pm) partitions) ->
  AllToAll back -> mobius matmul -> inv stages
so DVE/GPSIMD butterflies, TensorE matmuls and the collectives overlap.
"""
import sys
sys.path.insert(0, '/opt/trn_rl_repo')
import numpy as np

NCORES = 8
P = 128
F = 16384
BLK = 512            # matmul / block granularity
NBLK = F // BLK      # 32
NSLAB = 4
MSL = NBLK // NSLAB  # 8 blocks per slab
SLAB = F // NSLAB    # 4096 contiguous


def _zeta_mat(nbits):
    idx = np.arange(1 << nbits)
    return ((idx[:, None] & idx[None, :]) == idx[None, :]).astype(np.float32)


def _mobius_mat(nbits):
    idx = np.arange(1 << nbits)
    sup = (idx[:, None] & idx[None, :]) == idx[None, :]
    pc = np.array([bin(x).count("1") for x in range(1 << nbits)])
    signs = (-1.0) ** pc[idx[:, None] & ~idx[None, :]]
    return (sup * signs).astype(np.float32)


def build_kernel():
    import concourse.bacc as bacc
    import concourse.tile as tile
    from concourse import mybir

    f32 = mybir.dt.float32
    nc = bacc.Bacc("TRN2", target_bir_lowering=False, debug=False, num_devices=NCORES)

    m1_in = nc.dram_tensor("m1", [P, F], f32, kind="ExternalInput")
    m2_in = nc.dram_tensor("m2", [P, F], f32, kind="ExternalInput")
    out_t = nc.dram_tensor("out", [P, F], f32, kind="ExternalOutput")

    WZ7_d = nc.inline_tensor(_zeta_mat(7), name="WZ7")
    WM7_d = nc.inline_tensor(_mobius_mat(7), name="WM7")
    WZ3_d = nc.inline_tensor(np.kron(_zeta_mat(3), np.eye(16, dtype=np.float32)), name="WZ3x")
    WM3_d = nc.inline_tensor(np.kron(_mobius_mat(3), np.eye(16, dtype=np.float32)), name="WM3x")

    with tile.TileContext(nc) as tc:
        with tc.tile_pool(name="sbuf", bufs=1) as pool, \
             tc.tile_pool(name="chunks", bufs=4) as cpool, \
             tc.tile_pool(name="psum", bufs=2, space="PSUM") as psum, \
             tc.tile_pool(name="dram", bufs=1, space="DRAM") as dram:

            wz7 = pool.tile([P, P], f32)
            wm7 = pool.tile([P, P], f32)
            wz3 = pool.tile([P, P], f32)
            wm3 = pool.tile([P, P], f32)
            nc.sync.dma_start(out=wz7[:], in_=WZ7_d[:])
            nc.sync.dma_start(out=wm7[:], in_=WM7_d[:])
            nc.sync.dma_start(out=wz3[:], in_=WZ3_d[:])
            nc.sync.dma_start(out=wm3[:], in_=WM3_d[:])

            A = pool.tile([P, F], f32)
            B = pool.tile([P, F], f32)
            nc.sync.dma_start(out=A[:], in_=m1_in[:])
            nc.scalar.dma_start(out=B[:], in_=m2_in[:])

            cc_in = [dram.tile([NCORES, 2, 16, MSL * BLK], f32, tag=f"cci{s}", name=f"cci{s}") for s in range(NSLAB)]
            cc_out = [dram.tile([NCORES, 2, 16, MSL * BLK], f32, tag=f"cco{s}", name=f"cco{s}") for s in range(NSLAB)]
            cc2_in = [dram.tile([NCORES, 16, MSL * BLK], f32, tag=f"c2i{s}", name=f"c2i{s}") for s in range(NSLAB)]
            cc2_out = [dram.tile([NCORES, 16, MSL * BLK], f32, tag=f"c2o{s}", name=f"c2o{s}") for s in range(NSLAB)]

            ADD = "add"

            def tt(eng, alu, lo, hi):
                f = getattr(nc.vector if eng == "v" else nc.gpsimd,
                            "tensor_add" if alu == ADD else "tensor_sub")
                f(lo, lo, hi)

            def bf_split(alu, lo, hi):
                # DVE only: GPSIMD shares SBUF ports with DVE, so
                # concurrent fp32 TT on both engines halves each rate.
                tt("v", alu, lo, hi)

            def s13(t, alu):
                # bit-13 butterfly: cols [0:8k] (+/-)= cols [8k:16k]
                tt("v", alu, t[:, 0:8192], t[:, 8192:16384])

            def s12a(t, alu):
                tt("v", alu, t[:, 0:4096], t[:, 4096:8192])

            def s12b(t, alu):
                tt("v", alu, t[:, 8192:12288], t[:, 12288:16384])

            def slab_stages(t, s, alu):
                # intra-slab stages j=0..11 on contiguous slab s
                sl = t[:, s * SLAB:(s + 1) * SLAB]
                for j in range(12):
                    w = sl.rearrange("p (a two b) -> p a two b", two=2, b=1 << j)
                    bf_split(alu, w[:, :, 0, :], w[:, :, 1, :])

            def mm_block(t, w, blk):
                ps = psum.tile([P, BLK], f32, tag="ps_mm")
                sl = t[:, blk * BLK:(blk + 1) * BLK]
                nc.tensor.matmul(ps[:], lhsT=w[:], rhs=sl, start=True, stop=True)
                nc.scalar.copy(sl, ps[:])

            def slab_ap(t, s, prange=None):
                if prange is None:
                    return t[:, s * SLAB:(s + 1) * SLAB]
                return t[prange[0]:prange[1], s * SLAB:(s + 1) * SLAB]

            def emit_phase3_back(s):
                # recv, h-conv, back-send, back A2A for slab s
                for c in range(NCORES):
                    nc.gpsimd.dma_start(out=slab_ap(A, s, (16 * c, 16 * (c + 1))), in_=cc_out[s][c, 0])
                    nc.gpsimd.dma_start(out=slab_ap(B, s, (16 * c, 16 * (c + 1))), in_=cc_out[s][c, 1])
                for m in range(MSL):
                    blk = s * MSL + m
                    sa = A[:, blk * BLK:(blk + 1) * BLK]
                    sb = B[:, blk * BLK:(blk + 1) * BLK]
                    psA = psum.tile([P, BLK], f32, tag="psA")
                    psB = psum.tile([P, BLK], f32, tag="psB")
                    nc.tensor.matmul(psA[:], lhsT=wz3[:], rhs=sa, start=True, stop=True)
                    nc.tensor.matmul(psB[:], lhsT=wz3[:], rhs=sb, start=True, stop=True)
                    qa = cpool.tile([P, BLK], f32, tag="qa")
                    nc.scalar.copy(qa[:], psA[:])
                    pr = cpool.tile([P, BLK], f32, tag="pr")
                    nc.vector.tensor_mul(pr[:], qa[:], psB[:])
                    psU = psum.tile([P, BLK], f32, tag="psU")
                    nc.tensor.matmul(psU[:], lhsT=wm3[:], rhs=pr[:], start=True, stop=True)
                    nc.scalar.copy(sa, psU[:])
                for c in range(NCORES):
                    nc.sync.dma_start(out=cc2_in[s][c], in_=slab_ap(A, s, (16 * c, 16 * (c + 1))))
                nc.gpsimd.collective_compute(
                    "AllToAll", mybir.AluOpType.bypass,
                    replica_groups=[list(range(NCORES))],
                    ins=[cc2_in[s][:].opt()], outs=[cc2_out[s][:].opt()],
                )

            # ---------- phase 1 + forward A2As, back path interleaved 2 behind ----------
            for t in (A, B):
                s13(t, ADD)
                s12a(t, ADD)
                s12b(t, ADD)
            for s in range(NSLAB):
                for t in (A, B):
                    slab_stages(t, s, ADD)
                    for m in range(MSL):
                        mm_block(t, wz7, s * MSL + m)
                for d in range(NCORES):
                    nc.sync.dma_start(out=cc_in[s][d, 0], in_=slab_ap(A, s, (16 * d, 16 * (d + 1))))
                    nc.sync.dma_start(out=cc_in[s][d, 1], in_=slab_ap(B, s, (16 * d, 16 * (d + 1))))
                nc.gpsimd.collective_compute(
                    "AllToAll", mybir.AluOpType.bypass,
                    replica_groups=[list(range(NCORES))],
                    ins=[cc_in[s][:].opt()], outs=[cc_out[s][:].opt()],
                )
                if s >= 2:
                    emit_phase3_back(s - 2)
            for s in (NSLAB - 2, NSLAB - 1):
                emit_phase3_back(s)

            # ---------- phase 5 per slab ----------
            for s in range(NSLAB):
                for d in range(NCORES):
                    nc.gpsimd.dma_start(out=slab_ap(B, s, (16 * d, 16 * (d + 1))), in_=cc2_out[s][d])
                for m in range(MSL):
                    mm_block(B, wm7, s * MSL + m)
                slab_stages(B, s, "sub")
                if s == 1:
                    # needs only slabs 0,1 — run while slabs 2,3 are in flight
                    s12a(B, "sub")

            # ---------- inverse cross-slab stages + streamed output ----------
            s12b(B, "sub")
            nc.sync.dma_start(out=out_t[:, 12288:16384], in_=B[:, 12288:16384])
            s13(B, "sub")
            nc.sync.dma_start(out=out_t[:, 8192:12288], in_=B[:, 8192:12288])
            nc.sync.dma_start(out=out_t[:, 0:8192], in_=B[:, 0:8192])

    nc.compile()
    return nc


_NC_CACHE = None


def kernel(m12: np.ndarray) -> np.ndarray:
    global _NC_CACHE
    from concourse.bass_utils import run_bass_kernel_spmd

    if _NC_CACHE is None:
        _NC_CACHE = build_kernel()
    nc = _NC_CACHE

    m12 = np.ascontiguousarray(np.asarray(m12, dtype=np.float32))
    Bsz, C, L = m12.shape
    S = L // NCORES
    in_maps = []
    for c in range(NCORES):
        in_maps.append({
            "m1": m12[0, 0, c * S:(c + 1) * S].reshape(P, F),
            "m2": m12[0, 1, c * S:(c + 1) * S].reshape(P, F),
        })
    try:
        res = run_bass_kernel_spmd(nc, in_maps, core_ids=list(range(NCORES)))
    except Exception:
        # transient NRT/device hiccups have been observed; retry once
        import time
        time.sleep(5)
        res = run_bass_kernel_spmd(nc, in_maps, core_ids=list(range(NCORES)))
    out = np.concatenate([res.results[c]["out"].reshape(-1) for c in range(NCORES)])
    return out.reshape(1, L, 1, 1)


if __name__ == "__main__":
    m12 = np.load("/root/problem/m12.npy")
    out = kernel(m12)
    exp = np.load("/root/problem/expected.npy")
    err = np.abs(out - exp).max()
    scale = np.abs(exp).max()
    print(f"absmax err {err:.4g} scale {scale:.4g} rel {err/scale:.3e}")


# revision 26
# speedup vs baseline: 1.0201x; 1.0201x over previous
"""Distributed AND-convolution (Dempster combination / FWHT-style) for 8 TRN2 cores.

out = mobius(zeta(m1) * zeta(m2)) over 24 bit-axes, L = 2^24.

Sharding: top 3 bits (h = k[23:21]) = core id. Per core per channel: 2^21
elements, SBUF (128, 16384): partition p = l[20:14], free f = l[13:0].

Slab pipelining: f-blocks of 512 are interleaved into 4 slabs
(slab s = blocks with blk%4 == s, blk = f>>9). Free-bit butterfly stages
j in {0..8, 11..13} are intra-slab; j in {9, 10} are cross-slab and run
full-tile at the start (fwd) / end (inv). Each slab flows independently:
  fwd stages -> zeta matmul (7 partition bits) -> AllToAll ->
  h-conv (zeta3/product/mobius3 matmuls on (c,pm) partitions) ->
  AllToAll back -> mobius matmul -> inv stages
so DVE/GPSIMD butterflies, TensorE matmuls and the collectives overlap.
"""
import sys
sys.path.insert(0, '/opt/trn_rl_repo')
import numpy as np

NCORES = 8
P = 128
F = 16384
BLK = 512            # matmul / block granularity
NBLK = F // BLK      # 32
NSLAB = 4
MSL = NBLK // NSLAB  # 8 blocks per slab
SLAB = F // NSLAB    # 4096 contiguous


def _zeta_mat(nbits):
    idx = np.arange(1 << nbits)
    return ((idx[:, None] & idx[None, :]) == idx[None, :]).astype(np.float32)


def _mobius_mat(nbits):
    idx = np.arange(1 << nbits)
    sup = (idx[:, None] & idx[None, :]) == idx[None, :]
    pc = np.array([bin(x).count("1") for x in range(1 << nbits)])
    signs = (-1.0) ** pc[idx[:, None] & ~idx[None, :]]
    return (sup * signs).astype(np.float32)


def build_kernel():
    import concourse.bacc as bacc
    import concourse.tile as tile
    from concourse import mybir

    f32 = mybir.dt.float32
    nc = bacc.Bacc("TRN2", target_bir_lowering=False, debug=False, num_devices=NCORES)

    m1_in = nc.dram_tensor("m1", [P, F], f32, kind="ExternalInput")
    m2_in = nc.dram_tensor("m2", [P, F], f32, kind="ExternalInput")
    out_t = nc.dram_tensor("out", [P, F], f32, kind="ExternalOutput")

    WZ7_d = nc.inline_tensor(_zeta_mat(7), name="WZ7")
    WM7_d = nc.inline_tensor(_mobius_mat(7), name="WM7")
    WZ3_d = nc.inline_tensor(np.kron(_zeta_mat(3), np.eye(16, dtype=np.float32)), name="WZ3x")
    WM3_d = nc.inline_tensor(np.kron(_mobius_mat(3), np.eye(16, dtype=np.float32)), name="WM3x")

    with tile.TileContext(nc) as tc:
        with tc.tile_pool(name="sbuf", bufs=1) as pool, \
             tc.tile_pool(name="chunks", bufs=4) as cpool, \
             tc.tile_pool(name="psum", bufs=2, space="PSUM") as psum, \
             tc.tile_pool(name="dram", bufs=1, space="DRAM") as dram:

            wz7 = pool.tile([P, P], f32)
            wm7 = pool.tile([P, P], f32)
            wz3 = pool.tile([P, P], f32)
            wm3 = pool.tile([P, P], f32)
            nc.sync.dma_start(out=wz7[:], in_=WZ7_d[:])
            nc.sync.dma_start(out=wm7[:], in_=WM7_d[:])
            nc.sync.dma_start(out=wz3[:], in_=WZ3_d[:])
            nc.sync.dma_start(out=wm3[:], in_=WM3_d[:])

            A = pool.tile([P, F], f32)
            B = pool.tile([P, F], f32)
            nc.sync.dma_start(out=A[:], in_=m1_in[:])
            nc.scalar.dma_start(out=B[:], in_=m2_in[:])

            cc_in = [dram.tile([NCORES, 2, 16, MSL * BLK], f32, tag=f"cci{s}", name=f"cci{s}") for s in range(NSLAB)]
            cc_out = [dram.tile([NCORES, 2, 16, MSL * BLK], f32, tag=f"cco{s}", name=f"cco{s}") for s in range(NSLAB)]
            cc2_in = [dram.tile([NCORES, 16, MSL * BLK], f32, tag=f"c2i{s}", name=f"c2i{s}") for s in range(NSLAB)]
            cc2_out = [dram.tile([NCORES, 16, MSL * BLK], f32, tag=f"c2o{s}", name=f"c2o{s}") for s in range(NSLAB)]

            ADD = "add"

            def tt(eng, alu, lo, hi):
                f = getattr(nc.vector if eng == "v" else nc.gpsimd,
                            "tensor_add" if alu == ADD else "tensor_sub")
                f(lo, lo, hi)

            def bf_split(alu, lo, hi):
                # DVE only: GPSIMD shares SBUF ports with DVE, so
                # concurrent fp32 TT on both engines halves each rate.
                tt("v", alu, lo, hi)

            def s13(t, alu):
                # bit-13 butterfly: cols [0:8k] (+/-)= cols [8k:16k]
                tt("v", alu, t[:, 0:8192], t[:, 8192:16384])

            def s12a(t, alu):
                tt("v", alu, t[:, 0:4096], t[:, 4096:8192])

            def s12b(t, alu):
                tt("v", alu, t[:, 8192:12288], t[:, 12288:16384])

            def slab_stages(t, s, alu):
                # intra-slab stages j=0..11 on contiguous slab s
                sl = t[:, s * SLAB:(s + 1) * SLAB]
                for j in range(12):
                    w = sl.rearrange("p (a two b) -> p a two b", two=2, b=1 << j)
                    bf_split(alu, w[:, :, 0, :], w[:, :, 1, :])

            def mm_block(t, w, blk):
                ps = psum.tile([P, BLK], f32, tag="ps_mm")
                sl = t[:, blk * BLK:(blk + 1) * BLK]
                nc.tensor.matmul(ps[:], lhsT=w[:], rhs=sl, start=True, stop=True)
                nc.scalar.copy(sl, ps[:])

            def slab_ap(t, s, prange=None):
                if prange is None:
                    return t[:, s * SLAB:(s + 1) * SLAB]
                return t[prange[0]:prange[1], s * SLAB:(s + 1) * SLAB]

            def emit_phase3_back(s):
                # recv, h-conv, back-send, back A2A for slab s
                for c in range(NCORES):
                    nc.gpsimd.dma_start(out=slab_ap(A, s, (16 * c, 16 * (c + 1))), in_=cc_out[s][c, 0])
                    nc.gpsimd.dma_start(out=slab_ap(B, s, (16 * c, 16 * (c + 1))), in_=cc_out[s][c, 1])
                for m in range(MSL):
                    blk = s * MSL + m
                    sa = A[:, blk * BLK:(blk + 1) * BLK]
                    sb = B[:, blk * BLK:(blk + 1) * BLK]
                    psA = psum.tile([P, BLK], f32, tag="psA")
                    psB = psum.tile([P, BLK], f32, tag="psB")
                    nc.tensor.matmul(psA[:], lhsT=wz3[:], rhs=sa, start=True, stop=True)
                    nc.tensor.matmul(psB[:], lhsT=wz3[:], rhs=sb, start=True, stop=True)
                    qa = cpool.tile([P, BLK], f32, tag="qa")
                    nc.scalar.copy(qa[:], psA[:])
                    pr = cpool.tile([P, BLK], f32, tag="pr")
                    nc.vector.tensor_mul(pr[:], qa[:], psB[:])
                    psU = psum.tile([P, BLK], f32, tag="psU")
                    nc.tensor.matmul(psU[:], lhsT=wm3[:], rhs=pr[:], start=True, stop=True)
                    nc.scalar.copy(sa, psU[:])
                for c in range(NCORES):
                    nc.sync.dma_start(out=cc2_in[s][c], in_=slab_ap(A, s, (16 * c, 16 * (c + 1))))
                nc.gpsimd.collective_compute(
                    "AllToAll", mybir.AluOpType.bypass,
                    replica_groups=[list(range(NCORES))],
                    ins=[cc2_in[s][:].opt()], outs=[cc2_out[s][:].opt()],
                )

            # ---------- phase 1 + forward A2As, back path interleaved 2 behind ----------
            for t in (A, B):
                s13(t, ADD)
                s12a(t, ADD)
            for s in range(NSLAB):
                if s == 2:
                    # s12b (writes [8k:12k], feeds slabs 2,3) deferred out of
                    # the slab-0 critical path
                    s12b(A, ADD)
                    s12b(B, ADD)
                for t in (A, B):
                    slab_stages(t, s, ADD)
                    for m in range(MSL):
                        mm_block(t, wz7, s * MSL + m)
                for d in range(NCORES):
                    nc.sync.dma_start(out=cc_in[s][d, 0], in_=slab_ap(A, s, (16 * d, 16 * (d + 1))))
                    nc.sync.dma_start(out=cc_in[s][d, 1], in_=slab_ap(B, s, (16 * d, 16 * (d + 1))))
                nc.gpsimd.collective_compute(
                    "AllToAll", mybir.AluOpType.bypass,
                    replica_groups=[list(range(NCORES))],
                    ins=[cc_in[s][:].opt()], outs=[cc_out[s][:].opt()],
                )
                if s >= 2:
                    emit_phase3_back(s - 2)
            for s in (NSLAB - 2, NSLAB - 1):
                emit_phase3_back(s)

            # ---------- phase 5 per slab ----------
            for s in range(NSLAB):
                for d in range(NCORES):
                    nc.gpsimd.dma_start(out=slab_ap(B, s, (16 * d, 16 * (d + 1))), in_=cc2_out[s][d])
                for m in range(MSL):
                    mm_block(B, wm7, s * MSL + m)
                slab_stages(B, s, "sub")
                if s == 1:
                    # needs only slabs 0,1 — run while slabs 2,3 are in flight
                    s12a(B, "sub")

            # ---------- inverse cross-slab stages + streamed output ----------
            s12b(B, "sub")
            nc.sync.dma_start(out=out_t[:, 12288:16384], in_=B[:, 12288:16384])
            s13(B, "sub")
            nc.sync.dma_start(out=out_t[:, 8192:12288], in_=B[:, 8192:12288])
            nc.sync.dma_start(out=out_t[:, 0:8192], in_=B[:, 0:8192])

    nc.compile()
    return nc


_NC_CACHE = None


def kernel(m12: np.ndarray) -> np.ndarray:
    global _NC_CACHE
    from concourse.bass_utils import run_bass_kernel_spmd

    if _NC_CACHE is None:
        _NC_CACHE = build_kernel()
    nc = _NC_CACHE

    m12 = np.ascontiguousarray(np.asarray(m12, dtype=np.float32))
    Bsz, C, L = m12.shape
    S = L // NCORES
    in_maps = []
    for c in range(NCORES):
        in_maps.append({
            "m1": m12[0, 0, c * S:(c + 1) * S].reshape(P, F),
            "m2": m12[0, 1, c * S:(c + 1) * S].reshape(P, F),
        })
    try:
        res = run_bass_kernel_spmd(nc, in_maps, core_ids=list(range(NCORES)))
    except Exception:
        # transient NRT/device hiccups have been observed; retry once
        import time
        time.sleep(5)
        res = run_bass_kernel_spmd(nc, in_maps, core_ids=list(range(NCORES)))
    out = np.concatenate([res.results[c]["out"].reshape(-1) for c in range(NCORES)])
    return out.reshape(1, L, 1, 1)


if __name__ == "__main__":
    m12 = np.load("/root/problem/m12.npy")
    out = kernel(m12)
    exp = np.load("/root/problem/expected.npy")
    err = np.abs(out - exp).max()
    scale = np.abs(exp).max()
    print(f"absmax err {err:.4g} scale {scale:.4g} rel {err/scale:.3e}")
